# revision 13
# baseline (speedup 1.0000x reference)
"""DimeNet++ forward pass on 8 Trainium2 NeuronCores.

Sharding: core c = (b = c//4, q = c%4); core (b, q) owns edges
[q*16384, (q+1)*16384) of batch b, reordered into "slots": sorted by
destination node i[e] into 8 slot-windows of 2048 (window w covers
nodes in [512w-256, 512w+768)), then LPT-balanced over 16 bins of 128
slots per window so triplet counts per 128-slot agg window are even.

Triplet scatter-sum: one-hot matmuls (S[slot, e] = (dst[slot] == e))
accumulated in PSUM per 128-edge agg window. Gather x_kj[idx_kj]: bf16
table of all 4 quarters' x_kj, pair-packed [32768, 128] so int16
dma_gather indices reach every row; rebuilt per block via AllGather.
Node scatter: one-hot matmuls into [128ch, 1024node] PSUM per slot
window, then ReduceScatter so each core runs the node MLP on its
quarter of the nodes.  Dense edge MLPs run as fp32r matmuls over
[ch, slot] layout with h resident in SBUF.
"""

import os
import time
import numpy as np
import ml_dtypes

import concourse.bass as bass
import concourse.mybir as mybir
import concourse.tile as tile
from concourse import bacc
from concourse import bass_utils
from concourse.masks import make_identity

F32 = mybir.dt.float32
F32R = mybir.dt.float32r
BF16 = mybir.dt.bfloat16
I16 = mybir.dt.int16

B, N, E, T = 2, 4096, 65536, 524288
HC, OC, NB, IE, BE, NS, NR, OE = 128, 1, 4, 64, 8, 7, 6, 256
CUTOFF = 5.0
P_ENV = 6
EC = E // 4            # edges per core
NWIN = EC // 128       # agg windows per core
SW = 8                 # slot windows (2048 slots each)
NT = 16                # 128-slot tiles per slot window
NCH = EC // 512        # dense chunks of 512

bf16 = ml_dtypes.bfloat16

SILU = mybir.ActivationFunctionType.Silu
EQ = mybir.AluOpType.is_equal
MUL = mybir.AluOpType.mult
SUB = mybir.AluOpType.subtract
DBG = os.environ.get("KDBG", "")
PHASE = os.environ.get("KPHASE", "full")
NOGATHER = os.environ.get("KNOGATHER", "")


# ----------------------------------------------------------------------------
# host preprocessing
# ----------------------------------------------------------------------------

def _np(t):
    return np.asarray(t)


def _host_plan(x, dist, sbf, i, j, idx_kj, idx_ji, params):
    i = _np(i).astype(np.int64)
    j = _np(j).astype(np.int64)
    idx_kj = _np(idx_kj).astype(np.int64)
    idx_ji = _np(idx_ji).astype(np.int64)
    x = _np(x).astype(np.float32)
    dist = _np(dist).astype(np.float32)
    sbf = _np(sbf).astype(np.float32)

    trip_cnt = np.bincount(idx_ji, minlength=E)

    # --- edge slot assignment per core quarter ---
    slot_of = np.empty(E, np.int64)
    edge_at = np.empty((4, EC), np.int64)
    for q in range(4):
        eq = np.arange(q * EC, (q + 1) * EC)
        order = eq[np.argsort(i[eq], kind="stable")]
        final = np.empty(EC, np.int64)
        for w in range(SW):
            wed = order[2048 * w:2048 * (w + 1)]
            nds = i[wed]
            lo, hi = 512 * w - 256, 512 * w + 768
            assert nds.min() >= lo and nds.max() < hi, (q, w, nds.min(), nds.max())
            ws = trip_cnt[wed]
            srt = np.argsort(-ws, kind="stable")
            binw = np.zeros(NT)
            binfill = [[] for _ in range(NT)]
            for ei in srt:
                cand = min((binw[bi], bi) for bi in range(NT)
                           if len(binfill[bi]) < 128)[1]
                binfill[cand].append(wed[ei])
                binw[cand] += ws[ei]
            final[2048 * w:2048 * (w + 1)] = np.concatenate(
                [np.array(bq, np.int64) for bq in binfill])
        edge_at[q] = final
        slot_of[final] = np.arange(EC)

    pos_g = (np.arange(E) // EC) * EC + slot_of
    grow = (pos_g >> 1).astype(np.int64)
    gpar = (pos_g & 1).astype(np.float32)

    # --- triplet grouping per core quarter ---
    owner_t = idx_ji // EC
    per_core = []
    ktmax = 0
    for q in range(4):
        tl = np.where(owner_t == q)[0]
        s = slot_of[idx_ji[tl]]
        w = s >> 7
        o = np.argsort(w, kind="stable")
        tl, s, w = tl[o], s[o], w[o]
        cnts = np.bincount(w, minlength=NWIN)
        ktmax = max(ktmax, int(np.ceil(cnts.max() / 128)))
        per_core.append((tl, s, cnts))
    KT = ktmax
    TPW = KT * 128
    TPAD = NWIN * TPW

    plans = []
    for q in range(4):
        tl, s, cnts = per_core[q]
        trip_slot = np.full((NWIN, TPW), -1, np.int64)
        dstf = np.full((NWIN, TPW), -1.0, np.float32)
        offs = np.zeros(NWIN + 1, np.int64)
        offs[1:] = np.cumsum(cnts)
        for wi in range(NWIN):
            c = cnts[wi]
            trip_slot[wi, :c] = tl[offs[wi]:offs[wi + 1]]
            dstf[wi, :c] = (s[offs[wi]:offs[wi + 1]] & 127).astype(np.float32)
        valid = trip_slot >= 0
        src = np.where(valid, idx_kj[np.maximum(trip_slot, 0)], 0)
        growq = np.where(valid, grow[src], 0).astype(np.int16)
        parq = np.where(valid, gpar[src], 0.0).astype(np.float32)

        gw = np.zeros((NWIN, 128, KT * 8), np.int16)
        ii = np.arange(TPW)
        prow, col = ii % 16, ii // 16
        for g in range(8):
            gw[:, g * 16 + prow, col] = growq
        dst_pk = dstf.reshape(NWIN, KT, 128).transpose(0, 2, 1).copy()
        par_pk = parq.reshape(NWIN, KT, 128).transpose(0, 2, 1).astype(bf16)

        eseq = edge_at[q]
        dstN = np.empty((SW, 128, NT), np.float32)
        for w in range(SW):
            sl = eseq[2048 * w:2048 * (w + 1)].reshape(NT, 128)
            dstN[w] = (i[sl] - (512 * w - 256)).astype(np.float32).T
        plans.append(dict(trip_slot=trip_slot, gw=gw, dst_pk=dst_pk,
                          par_pk=par_pk, dstN=dstN, eseq=eseq))

    p = params
    freq = _np(p["freq"]).astype(np.float32)
    a_env = -(P_ENV + 1) * (P_ENV + 2) / 2.0
    b_env = float(P_ENV * (P_ENV + 2))
    c_env = -P_ENV * (P_ENV + 1) / 2.0

    def W(lin):
        return _np(lin["W"]).astype(np.float32)

    def bvec(lin):
        return _np(lin["b"]).astype(np.float32)[:, None]

    pe = p["emb"]
    Wlin = W(pe["lin"])
    Wemb = W(pe["emb"])
    bemb = _np(pe["emb"]["b"]).astype(np.float32)
    u_a, u_b = Wemb @ Wlin[0:128], Wemb @ Wlin[128:256]      # [1,128]
    wts = dict(
        w_c=Wlin[256:384],
        b_comb=(bemb @ (Wlin[0:128] + Wlin[128:256])
                + _np(pe["lin"]["b"]).astype(np.float32))[:, None],
        w_rbfe=W(pe["rbf"]), b_rbfe=bvec(pe["rbf"]),
    )
    for l, blk in enumerate(p["blocks"]):
        wts[f"b{l}_wji"] = W(blk["ji"]); wts[f"b{l}_bji"] = bvec(blk["ji"])
        wts[f"b{l}_wkj"] = W(blk["kj"]); wts[f"b{l}_bkj"] = bvec(blk["kj"])
        wts[f"b{l}_w12r"] = W(blk["rbf1"]) @ W(blk["rbf2"])
        wts[f"b{l}_w12s"] = (W(blk["sbf1"]) @ W(blk["sbf2"])).astype(bf16)
        wts[f"b{l}_wdn"] = W(blk["down"])
        wts[f"b{l}_wup"] = W(blk["up"])
        wts[f"b{l}_wl1"] = W(blk["before"][0]["l1"]); wts[f"b{l}_bl1"] = bvec(blk["before"][0]["l1"])
        wts[f"b{l}_wl2"] = W(blk["before"][0]["l2"]); wts[f"b{l}_bl2"] = bvec(blk["before"][0]["l2"])
        wts[f"b{l}_wa1"] = W(blk["after"][0]["l1"]); wts[f"b{l}_ba1"] = bvec(blk["after"][0]["l1"])
        wts[f"b{l}_wa2"] = W(blk["after"][0]["l2"]); wts[f"b{l}_ba2"] = bvec(blk["after"][0]["l2"])
        wts[f"b{l}_wli"] = W(blk["lin"]); wts[f"b{l}_bli"] = bvec(blk["lin"])
    for o, ob in enumerate(p["outs"]):
        wts[f"o{o}_wr"] = W(ob["rbf"])
        wts[f"o{o}_wup"] = W(ob["up"])
        for k, lq in enumerate(ob["lins"]):
            wts[f"o{o}_wl{k}"] = W(lq)
            wts[f"o{o}_bl{k}"] = bvec(lq)
        wts[f"o{o}_wf"] = W(ob["lin"])

    in_maps = []
    for c in range(8):
        b, q = c // 4, c % 4
        pl = plans[q]
        eseq = pl["eseq"]
        d = dist[b, eseq] / CUTOFF
        env = 1.0 / d + a_env * d ** 5 + b_env * d ** 6 + c_env * d ** 7
        rbf = env[:, None] * np.sin(freq[None, :] * d[:, None])   # [EC, 6]
        x_i = x[b, i[eseq], 0]
        x_j = x[b, j[eseq], 0]
        h0pre = (u_a.T @ x_i[None, :] + u_b.T @ x_j[None, :])     # [128, EC]
        m = dict(
            rbfT=np.ascontiguousarray(rbf.T).astype(np.float32),
            h0pre=h0pre.astype(np.float32),
            gidx=pl["gw"].copy(),
            dstT=pl["dst_pk"].copy(),
            parT=pl["par_pk"].copy(),
            dstN=pl["dstN"].copy(),
            iota=np.broadcast_to(
                np.arange(1024, dtype=np.float32), (128, 1024)).copy(),
        )
        tv = pl["trip_slot"].reshape(-1)
        sb = np.zeros((TPAD, NS * NR), np.float32)
        msk = tv >= 0
        sb[msk] = sbf[b, tv[msk]]
        m["sbfT"] = np.ascontiguousarray(sb.T).astype(bf16)
        m.update(wts)
        in_maps.append(m)

    meta = dict(KT=KT, TPW=TPW, TPAD=TPAD, edge_at=edge_at, slot_of=slot_of,
                plans=plans)
    return in_maps, meta


# ----------------------------------------------------------------------------
# device program
# ----------------------------------------------------------------------------

def _build(KT, part):
    TPW = KT * 128
    TPAD = NWIN * TPW
    nc = bacc.Bacc("TRN2", target_bir_lowering=False, debug=False,
                   num_devices=8)

    def din(name, shape, dt):
        return nc.dram_tensor(name, shape, dt, kind="ExternalInput").ap()

    sbfT = din("sbfT", [NS * NR, TPAD], BF16)
    gidx = din("gidx", [NWIN, 128, KT * 8], I16)
    dstT = din("dstT", [NWIN, 128, KT], F32)
    parT = din("parT", [NWIN, 128, KT], BF16)
    dstN = din("dstN", [SW, 128, NT], F32)
    rbfT = din("rbfT", [NR, EC], F32R)
    if part == 1:
        h0pre = din("h0pre", [HC, EC], F32)
    iota = din("iota", [128, 1024], F32)

    wd = {}
    for nm, shape, dt in [
        ("w_c", [HC, HC], F32R), ("b_comb", [HC, 1], F32),
        ("w_rbfe", [NR, HC], F32R), ("b_rbfe", [HC, 1], F32),
    ]:
        wd[nm] = din(nm, shape, dt)
    for l in range(NB):
        for nm, shape, dt in [
            ("wji", [HC, HC], F32R), ("bji", [HC, 1], F32),
            ("wkj", [HC, HC], F32R), ("bkj", [HC, 1], F32),
            ("w12r", [NR, HC], F32R), ("w12s", [NS * NR, IE], BF16),
            ("wdn", [HC, IE], F32R), ("wup", [IE, HC], F32R),
            ("wl1", [HC, HC], F32R), ("bl1", [HC, 1], F32),
            ("wl2", [HC, HC], F32R), ("bl2", [HC, 1], F32),
            ("wa1", [HC, HC], F32R), ("ba1", [HC, 1], F32),
            ("wa2", [HC, HC], F32R), ("ba2", [HC, 1], F32),
            ("wli", [HC, HC], F32R), ("bli", [HC, 1], F32),
        ]:
            wd[f"b{l}_{nm}"] = din(f"b{l}_{nm}", shape, dt)
    for o in range(NB + 1):
        wd[f"o{o}_wr"] = din(f"o{o}_wr", [NR, HC], F32R)
        wd[f"o{o}_wup"] = din(f"o{o}_wup", [HC, OE], F32R)
        for k in range(3):
            wd[f"o{o}_wl{k}"] = din(f"o{o}_wl{k}", [OE, OE], F32R)
            wd[f"o{o}_bl{k}"] = din(f"o{o}_bl{k}", [OE, 1], F32)
        wd[f"o{o}_wf"] = din(f"o{o}_wf", [OE, 1], F32R)

    if part == 1:
        h_out = nc.dram_tensor("h_out", [HC, EC], F32R,
                               kind="ExternalOutput").ap()
        node_out = nc.dram_tensor("node_out", [4, 3, HC, 1024], F32,
                                  kind="ExternalOutput").ap()
    else:
        h_in = nc.dram_tensor("h_in", [HC, EC], F32R,
                              kind="ExternalInput").ap()
        nacc_in = nc.dram_tensor("nacc_in", [4, 3, HC, 1024], F32,
                                 kind="ExternalInput").ap()
        P_out = nc.dram_tensor("P_out", [1, 1024], F32,
                               kind="ExternalOutput").ap()
    dbg_out = None
    if DBG:
        dbg_out = nc.dram_tensor("dbg", [HC, EC], F32R,
                                 kind="ExternalOutput").ap()

    _lr = range(0, 2) if part == 1 else range(2, 4)
    xkj_bnc = {l: nc.dram_tensor(f"xkjb{l}", [EC, IE], BF16,
                                 kind="Internal").ap() for l in _lr}
    tables = {l: nc.dram_tensor(f"tab{l}", [E // 2, 2 * IE], BF16,
                                kind="Internal").ap() for l in _lr}
    if part == 1:
        node_bnc = node_out
    else:
        node_bnc = nc.dram_tensor("nodeball", [4, NB + 1, HC, 1024], F32,
                                  kind="Internal").ap()
        rs_out = nc.dram_tensor("rsall", [NB + 1, HC, 1024], F32,
                                kind="Internal").ap()

    GROUPS = [[0, 1, 2, 3], [4, 5, 6, 7]]

    with tile.TileContext(nc) as tc, \
         tc.tile_pool(name="cst", bufs=1) as cst, \
         tc.tile_pool(name="big", bufs=1) as big, \
         tc.tile_pool(name="mlp", bufs=1) as mlp, \
         tc.tile_pool(name="ow", bufs=1) as ow, \
         tc.tile_pool(name="bwp", bufs=2) as bwp, \
         tc.tile_pool(name="wk", bufs=2) as wk, \
         tc.tile_pool(name="tp", bufs=2) as tp, \
         tc.tile_pool(name="ps", bufs=2, space="PSUM") as ps, \
         tc.tile_pool(name="pst_p", bufs=1, space="PSUM") as pst_p, \
         tc.tile_pool(name="psg_p", bufs=1, space="PSUM") as psg_p, \
         tc.tile_pool(name="psx_p", bufs=1, space="PSUM") as psx_p:

        iota_t = cst.tile([128, 1024], F32)
        nc.sync.dma_start(iota_t[:], iota[:])
        ident = cst.tile([128, 128], F32)
        make_identity(nc, ident[:])
        ident_bf = cst.tile([128, 128], BF16)
        nc.vector.tensor_copy(ident_bf[:], ident[:])

        wt = {}
        for nm, ap in wd.items():
            if nm.startswith("o") or (nm[0] == "b" and nm[1].isdigit()):
                continue
            t = cst.tile(list(ap.shape), ap.dtype, tag=f"w_{nm}")
            nc.sync.dma_start(t[:], ap[:])
            wt[nm] = t
        wd_b = {nm: ap for nm, ap in wd.items()
                if nm[0] == "b" and nm[1].isdigit()}

        def load_block_weights(l):
            bw = {}
            for nm, ap in wd_b.items():
                if not nm.startswith(f"b{l}_"):
                    continue
                short = nm.split("_", 1)[1]
                t = bwp.tile(list(ap.shape), ap.dtype, tag=f"bw_{short}")
                nc.sync.dma_start(t[:], ap[:])
                bw[nm] = t
            return bw

        h = big.tile([128, EC], F32R, tag="h")
        node_ext = big.tile([128, 4608], F32, tag="node_ext")
        if part == 2:
            P_acc = big.tile([1, 1024], F32, tag="P_acc")
            nc.gpsimd.memset(P_acc[:], 0.0)

        def cs(c):
            return slice(512 * c, 512 * (c + 1))

        def load_rbf(c):
            r = wk.tile([NR, 512], F32R, tag="rbf_c")
            nc.sync.dma_start(r[:], rbfT[:, cs(c)])
            return r

        # ---- embedding (part 1) / h load (part 2) ----
        if part == 2:
            nc.sync.dma_start(h[:], h_in[:])
        for c in range(NCH if part == 1 else 0):
            rbc = load_rbf(c)
            psr = ps.tile([128, 512], F32, tag="psA")
            nc.tensor.matmul(psr[:], wt["w_rbfe"][:], rbc[:],
                             start=True, stop=True)
            rh = wk.tile([128, 512], F32R, tag="pA")
            nc.scalar.activation(rh[:], psr[:], SILU, bias=wt["b_rbfe"][:])
            ps2 = ps.tile([128, 512], F32, tag="psB")
            nc.tensor.matmul(ps2[:], wt["w_c"][:], rh[:],
                             start=True, stop=True)
            h0c = wk.tile([128, 512], F32, tag="pB")
            nc.sync.dma_start(h0c[:], h0pre[:, cs(c)])
            tsum = wk.tile([128, 512], F32, tag="pC")
            nc.vector.tensor_add(tsum[:], h0c[:], ps2[:])
            nc.scalar.activation(h[:, cs(c)], tsum[:], SILU,
                                 bias=wt["b_comb"][:])

        # ---- output block ----
        def out_block(o):
            own = {}
            t = ow.tile([NR, HC], F32R, tag="ow_wr")
            nc.sync.dma_start(t[:], wd[f"o{o}_wr"][:])
            own["wr"] = t
            nc.gpsimd.memset(node_ext[:], 0.0)
            for w in range(SW):
                nx = psx_p.tile([128, 1024], F32, tag="psX")
                na = nx[:, 0:512]
                nb_ = nx[:, 512:1024]
                dn = wk.tile([128, NT], F32, tag="dn")
                nc.sync.dma_start(dn[:], dstN[w])
                for c4 in range(4):
                    c = 4 * w + c4
                    rbc = load_rbf(c)
                    pso = ps.tile([128, 512], F32, tag="psA")
                    nc.tensor.matmul(pso[:], own["wr"][:], rbc[:],
                                     start=True, stop=True)
                    tb = wk.tile([128, 512], F32, tag="pD")
                    nc.vector.tensor_tensor(tb[:], h[:, cs(c)], pso[:], op=MUL)
                    for k4 in range(4):
                        k = 4 * c4 + k4
                        pst = pst_p.tile([128, 128], F32, tag="psT")
                        nc.tensor.transpose(
                            out=pst[:], in_=tb[:, 128 * k4:128 * (k4 + 1)],
                            identity=ident[:])
                        tt = wk.tile([128, 128], F32R, tag="tt")
                        nc.vector.tensor_copy(tt[:], pst[:])
                        Sn = wk.tile([128, 1024], F32R, tag="Sn")
                        nc.vector.tensor_tensor(
                            Sn[:], dn[:, k:k + 1].to_broadcast([128, 1024]),
                            iota_t[:], op=EQ)
                        nc.tensor.matmul(na, tt[:], Sn[:, 0:512],
                                         start=(k == 0), stop=(k == NT - 1))
                        nc.tensor.matmul(nb_, tt[:], Sn[:, 512:1024],
                                         start=(k == 0), stop=(k == NT - 1))
                nc.vector.tensor_add(node_ext[:, 512 * w:512 * (w + 1)],
                                     node_ext[:, 512 * w:512 * (w + 1)], na)
                nc.vector.tensor_add(node_ext[:, 512 * (w + 1):512 * (w + 2)],
                                     node_ext[:, 512 * (w + 1):512 * (w + 2)],
                                     nb_)
            for qq in range(4):
                nc.sync.dma_start(
                    node_bnc[qq, o],
                    node_ext[:, 256 + 1024 * qq:256 + 1024 * (qq + 1)])

        def mlp_block(o):
            own = {}
            for nm, shape in [("wup", [HC, OE]),
                              ("wl0", [OE, OE]), ("bl0", [OE, 1]),
                              ("wl1", [OE, OE]), ("bl1", [OE, 1]),
                              ("wl2", [OE, OE]), ("bl2", [OE, 1]),
                              ("wf", [OE, 1])]:
                ap = wd[f"o{o}_{nm}"]
                if shape[0] > 128:
                    t = ow.tile([128, 2] + shape[1:], ap.dtype, tag=f"ow_{nm}")
                    nc.sync.dma_start(
                        t[:], ap[:].rearrange("(a p) x -> p a x", p=128))
                else:
                    t = ow.tile(shape, ap.dtype, tag=f"ow_{nm}")
                    nc.sync.dma_start(t[:], ap[:])
                own[nm] = t
            u0r = mlp.tile([128, 1024], F32, tag="u0r")
            nc.sync.dma_start(u0r[:], rs_out[o])
            u0 = mlp.tile([128, 1024], F32R, tag="u0")
            nc.vector.tensor_copy(u0[:], u0r[:])
            u1 = mlp.tile([128, 2, 1024], F32R, tag="uA")
            for mo in range(2):
                for nn in range(2):
                    psu = ps.tile([128, 512], F32, tag="psA")
                    nc.tensor.matmul(psu[:],
                                     own["wup"][:, mo * 128:(mo + 1) * 128],
                                     u0[:, 512 * nn:512 * (nn + 1)],
                                     start=True, stop=True)
                    nc.vector.tensor_copy(u1[:, mo, 512 * nn:512 * (nn + 1)],
                                          psu[:])
            uprev = u1
            for k in range(3):
                unext = mlp.tile([128, 2, 1024], F32R,
                                 tag="uB" if k % 2 == 0 else "uA")
                wl = own[f"wl{k}"]
                bl = own[f"bl{k}"]
                for mo in range(2):
                    for nn in range(2):
                        psu = ps.tile([128, 512], F32, tag="psA")
                        nc.tensor.matmul(
                            psu[:], wl[:, 0, mo * 128:(mo + 1) * 128],
                            uprev[:, 0, 512 * nn:512 * (nn + 1)],
                            start=True, stop=False)
                        nc.tensor.matmul(
                            psu[:], wl[:, 1, mo * 128:(mo + 1) * 128],
                            uprev[:, 1, 512 * nn:512 * (nn + 1)],
                            start=False, stop=True)
                        nc.scalar.activation(
                            unext[:, mo, 512 * nn:512 * (nn + 1)], psu[:],
                            SILU, bias=bl[:, mo, :])
                uprev = unext
            wf = own["wf"]
            for nn in range(2):
                psf = psg_p.tile([1, 512], F32, tag="psG")
                nc.tensor.matmul(psf[:], wf[:, 0, :],
                                 uprev[:, 0, 512 * nn:512 * (nn + 1)],
                                 start=True, stop=False)
                nc.tensor.matmul(psf[:], wf[:, 1, :],
                                 uprev[:, 1, 512 * nn:512 * (nn + 1)],
                                 start=False, stop=True)
                nc.vector.tensor_add(P_acc[:, 512 * nn:512 * (nn + 1)],
                                     P_acc[:, 512 * nn:512 * (nn + 1)],
                                     psf[:])

        if part == 1:
            out_block(0)

        # ---- interaction blocks ----
        for l in (range(0, 2) if part == 1 else range(2, 4)):
            pre = f"b{l}_"
            wt_b = load_block_weights(l)
            wt.update(wt_b)
            for c in range(NCH):
                psk = ps.tile([128, 512], F32, tag="psA")
                nc.tensor.matmul(psk[:], wt[pre + "wkj"][:], h[:, cs(c)],
                                 start=True, stop=True)
                xkj = wk.tile([128, 512], F32R, tag="pA")
                nc.scalar.activation(xkj[:], psk[:], SILU,
                                     bias=wt[pre + "bkj"][:])
                rbc = load_rbf(c)
                psr = ps.tile([128, 512], F32, tag="psB")
                nc.tensor.matmul(psr[:], wt[pre + "w12r"][:], rbc[:],
                                 start=True, stop=True)
                xkr = wk.tile([128, 512], F32R, tag="pB")
                nc.vector.tensor_tensor(xkr[:], xkj[:], psr[:], op=MUL)
                psd = ps.tile([64, 512], F32, tag="psA")
                nc.tensor.matmul(psd[:], wt[pre + "wdn"][:], xkr[:],
                                 start=True, stop=True)
                xd = wk.tile([64, 512], BF16, tag="xd")
                nc.scalar.activation(xd[:], psd[:], SILU)
                pst = pst_p.tile([128, 256], BF16, tag="psT")
                for k in range(4):
                    nc.tensor.transpose(
                        out=pst[:, 64 * k:64 * (k + 1)],
                        in_=xd[:, 128 * k:128 * (k + 1)],
                        identity=ident_bf[:64, :64])
                tb4 = wk.tile([128, 4, 64], BF16, tag="tb4")
                nc.vector.tensor_copy(
                    tb4[:], pst[:].rearrange("p (a x) -> p a x", a=4))
                for a in range(4):
                    nc.sync.dma_start(
                        xkj_bnc[l][512 * c + 128 * a:512 * c + 128 * (a + 1), :],
                        tb4[:, a, :])
            nc.gpsimd.collective_compute(
                "AllGather", mybir.AluOpType.bypass, replica_groups=GROUPS,
                ins=[xkj_bnc[l].opt()], outs=[tables[l].opt()])

            agg_c = None
            for w in range(NWIN):
                gix = tp.tile([128, KT * 8], I16, tag="gix")
                nc.sync.dma_start(gix[:], gidx[w])
                dt_ = tp.tile([128, KT], F32, tag="dt_")
                nc.sync.dma_start(dt_[:], dstT[w])
                pt_ = tp.tile([128, KT], BF16, tag="pt_")
                nc.sync.dma_start(pt_[:], parT[w])
                sfl = tp.tile([NS * NR, TPW], BF16, tag="sfl")
                nc.sync.dma_start(sfl[:], sbfT[:, TPW * w:TPW * (w + 1)])
                gt = tp.tile([128, KT, 2 * IE], BF16, tag="gt")
                if NOGATHER:
                    nc.gpsimd.memset(gt[:], 0.0)
                else:
                    nc.gpsimd.dma_gather(
                        out_ap=gt[:], in_ap=tables[l][:], idxs_ap=gix[:],
                        num_idxs=TPW, num_idxs_reg=TPW, elem_size=2 * IE,
                        single_packet=False)
                psex = psx_p.tile([128, 1024], F32, tag="psX")
                pse = psex[:, 0:KT * IE]
                for k in range(KT):
                    nc.tensor.matmul(psex[:, IE * k:IE * (k + 1)],
                                     sfl[:, 128 * k:128 * (k + 1)],
                                     wt[pre + "w12s"][:],
                                     start=True, stop=True)
                sfe = tp.tile([128, KT, IE], BF16, tag="sfe")
                nc.vector.tensor_copy(
                    sfe[:], pse.rearrange("p (a x) -> p a x", a=KT))
                gd = tp.tile([128, KT, IE], BF16, tag="gd")
                nc.vector.tensor_tensor(gd[:], gt[:, :, IE:2 * IE],
                                        gt[:, :, 0:IE], op=SUB)
                nc.vector.tensor_tensor(
                    gd[:], gd[:],
                    pt_[:, :, None].to_broadcast([128, KT, IE]), op=MUL)
                nc.vector.tensor_add(gd[:], gd[:], gt[:, :, 0:IE])
                nc.vector.tensor_tensor(gd[:], gd[:], sfe[:], op=MUL)
                Sa = tp.tile([128, KT, 128], BF16, tag="Sa")
                nc.vector.tensor_tensor(
                    Sa[:], dt_[:, :, None].to_broadcast([128, KT, 128]),
                    iota_t[:, None, 0:128].to_broadcast([128, KT, 128]),
                    op=EQ)
                pag = psg_p.tile([64, 128], F32, tag="psG")
                for k in range(KT):
                    nc.tensor.matmul(pag[:], gd[:, k, :], Sa[:, k, :],
                                     start=(k == 0), stop=(k == KT - 1))
                if w % 4 == 0:
                    agg_c = wk.tile([64, 512], F32R, tag="agg_c")
                nc.vector.tensor_copy(
                    agg_c[:, 128 * (w % 4):128 * (w % 4 + 1)], pag[:])
                if w % 4 == 3:
                    c = w // 4
                    psu = ps.tile([128, 512], F32, tag="psA")
                    nc.tensor.matmul(psu[:], wt[pre + "wup"][:], agg_c[:],
                                     start=True, stop=True)
                    t1 = wk.tile([128, 512], F32R, tag="pA")
                    nc.scalar.activation(t1[:], psu[:], SILU)
                    psj = ps.tile([128, 512], F32, tag="psB")
                    nc.tensor.matmul(psj[:], wt[pre + "wji"][:], h[:, cs(c)],
                                     start=True, stop=True)
                    t2 = wk.tile([128, 512], F32R, tag="pB")
                    nc.scalar.activation(t2[:], psj[:], SILU,
                                         bias=wt[pre + "bji"][:])
                    hh = wk.tile([128, 512], F32R, tag="pC")
                    nc.vector.tensor_add(hh[:], t1[:], t2[:])
                    ps1 = ps.tile([128, 512], F32, tag="psA")
                    nc.tensor.matmul(ps1[:], wt[pre + "wl1"][:], hh[:],
                                     start=True, stop=True)
                    r1 = wk.tile([128, 512], F32R, tag="pA")
                    nc.scalar.activation(r1[:], ps1[:], SILU,
                                         bias=wt[pre + "bl1"][:])
                    ps2_ = ps.tile([128, 512], F32, tag="psB")
                    nc.tensor.matmul(ps2_[:], wt[pre + "wl2"][:], r1[:],
                                     start=True, stop=True)
                    r2 = wk.tile([128, 512], F32R, tag="pB")
                    nc.scalar.activation(r2[:], ps2_[:], SILU,
                                         bias=wt[pre + "bl2"][:])
                    nc.vector.tensor_add(hh[:], hh[:], r2[:])
                    ps3 = ps.tile([128, 512], F32, tag="psA")
                    nc.tensor.matmul(ps3[:], wt[pre + "wli"][:], hh[:],
                                     start=True, stop=True)
                    r3 = wk.tile([128, 512], F32R, tag="pA")
                    nc.scalar.activation(r3[:], ps3[:], SILU,
                                         bias=wt[pre + "bli"][:])
                    nc.vector.tensor_add(h[:, cs(c)], h[:, cs(c)], r3[:])
                    ps4 = ps.tile([128, 512], F32, tag="psB")
                    nc.tensor.matmul(ps4[:], wt[pre + "wa1"][:], h[:, cs(c)],
                                     start=True, stop=True)
                    r4 = wk.tile([128, 512], F32R, tag="pB")
                    nc.scalar.activation(r4[:], ps4[:], SILU,
                                         bias=wt[pre + "ba1"][:])
                    ps5 = ps.tile([128, 512], F32, tag="psA")
                    nc.tensor.matmul(ps5[:], wt[pre + "wa2"][:], r4[:],
                                     start=True, stop=True)
                    r5 = wk.tile([128, 512], F32R, tag="pC2")
                    nc.scalar.activation(r5[:], ps5[:], SILU,
                                         bias=wt[pre + "ba2"][:])
                    nc.vector.tensor_add(h[:, cs(c)], h[:, cs(c)], r5[:])

            if DBG == f"h{l}":
                nc.sync.dma_start(dbg_out[:], h[:])
            out_block(l + 1)

        if DBG == "emb":
            nc.sync.dma_start(dbg_out[:], h[:])

        if part == 1:
            nc.sync.dma_start(h_out[:], h[:])
        else:
            for qq in range(4):
                for o in range(3):
                    stg = wk.tile([128, 1024], F32, tag="stg")
                    nc.sync.dma_start(stg[:], nacc_in[qq, o])
                    nc.sync.dma_start(node_bnc[qq, o], stg[:])
            nc.gpsimd.collective_compute(
                "ReduceScatter", mybir.AluOpType.add, replica_groups=GROUPS,
                ins=[node_bnc.opt()], outs=[rs_out.opt()])
            for o in range(NB + 1):
                mlp_block(o)
            nc.sync.dma_start(P_out[:], P_acc[:])

    t0 = time.time()
    nc.compile()
    print(f"[kernel] bacc compile {time.time() - t0:.1f}s", flush=True)
    return nc


# ----------------------------------------------------------------------------
# entry point
# ----------------------------------------------------------------------------

_CACHE = {}


def kernel(x, dist, sbf, i, j, idx_kj, idx_ji, params):
    t0 = time.time()
    in_maps, meta = _host_plan(x, dist, sbf, i, j, idx_kj, idx_ji, params)
    print(f"[kernel] host plan {time.time() - t0:.1f}s (KT={meta['KT']})",
          flush=True)
    KT = meta["KT"]
    for part in (1, 2):
        if (KT, part) not in _CACHE:
            t0 = time.time()
            _CACHE[(KT, part)] = _build(KT, part)
            print(f"[kernel] build part{part} {time.time() - t0:.1f}s",
                  flush=True)
    nc1, nc2 = _CACHE[(KT, 1)], _CACHE[(KT, 2)]
    t0 = time.time()
    res1 = bass_utils.run_bass_kernel_spmd(nc1, in_maps,
                                           core_ids=list(range(8)))
    print(f"[kernel] run part1 {time.time() - t0:.1f}s", flush=True)
    in_maps2 = []
    for c in range(8):
        m = dict(in_maps[c])
        m["h_in"] = res1.results[c]["h_out"]
        m["nacc_in"] = res1.results[c]["node_out"]
        m.pop("h0pre")
        in_maps2.append(m)
    t0 = time.time()
    res = bass_utils.run_bass_kernel_spmd(nc2, in_maps2,
                                          core_ids=list(range(8)))
    print(f"[kernel] run part2 {time.time() - t0:.1f}s", flush=True)
    P = np.zeros((B, N, OC), np.float32)
    for c in range(8):
        b, q = c // 4, c % 4
        P[b, 1024 * q:1024 * (q + 1), 0] = res.results[c]["P_out"][0]
    kernel._last = (nc1, nc2, in_maps, meta, res1, res)
    return P


# revision 15
# speedup vs baseline: 1.6015x; 1.6015x over previous
"""DimeNet++ forward pass on 8 Trainium2 NeuronCores.

Sharding: core c = (b = c//4, q = c%4); core (b, q) owns edges
[q*16384, (q+1)*16384) of batch b, reordered into "slots": sorted by
destination node i[e] into 8 slot-windows of 2048 (window w covers
nodes in [512w-256, 512w+768)), then LPT-balanced over 16 bins of 128
slots per window so triplet counts per 128-slot agg window are even.

Triplet scatter-sum: one-hot matmuls (S[slot, e] = (dst[slot] == e))
accumulated in PSUM per 128-edge agg window. Gather x_kj[idx_kj]: bf16
table of all 4 quarters' x_kj, pair-packed [32768, 128] so int16
dma_gather indices reach every row; rebuilt per block via AllGather.
Node scatter: one-hot matmuls into [128ch, 1024node] PSUM per slot
window, then ReduceScatter so each core runs the node MLP on its
quarter of the nodes.  Dense edge MLPs run as fp32r matmuls over
[ch, slot] layout with h resident in SBUF.
"""

import os
import time
import numpy as np
import ml_dtypes

import concourse.bass as bass
import concourse.mybir as mybir
import concourse.tile as tile
from concourse import bacc
from concourse import bass_utils
from concourse.masks import make_identity

F32 = mybir.dt.float32
F32R = mybir.dt.float32r
BF16 = mybir.dt.bfloat16
I16 = mybir.dt.int16

B, N, E, T = 2, 4096, 65536, 524288
HC, OC, NB, IE, BE, NS, NR, OE = 128, 1, 4, 64, 8, 7, 6, 256
CUTOFF = 5.0
P_ENV = 6
EC = E // 4            # edges per core
NWIN = EC // 128       # agg windows per core
SW = 8                 # slot windows (2048 slots each)
NT = 16                # 128-slot tiles per slot window
NCH = EC // 512        # dense chunks of 512

bf16 = ml_dtypes.bfloat16

SILU = mybir.ActivationFunctionType.Silu
EQ = mybir.AluOpType.is_equal
MUL = mybir.AluOpType.mult
SUB = mybir.AluOpType.subtract
DBG = os.environ.get("KDBG", "")
PHASE = os.environ.get("KPHASE", "full")
NOGATHER = os.environ.get("KNOGATHER", "")


# ----------------------------------------------------------------------------
# host preprocessing
# ----------------------------------------------------------------------------

def _np(t):
    return np.asarray(t)


def _host_plan(x, dist, sbf, i, j, idx_kj, idx_ji, params):
    i = _np(i).astype(np.int64)
    j = _np(j).astype(np.int64)
    idx_kj = _np(idx_kj).astype(np.int64)
    idx_ji = _np(idx_ji).astype(np.int64)
    x = _np(x).astype(np.float32)
    dist = _np(dist).astype(np.float32)
    sbf = _np(sbf).astype(np.float32)

    trip_cnt = np.bincount(idx_ji, minlength=E)

    # --- edge slot assignment per core quarter ---
    slot_of = np.empty(E, np.int64)
    edge_at = np.empty((4, EC), np.int64)
    for q in range(4):
        eq = np.arange(q * EC, (q + 1) * EC)
        order = eq[np.argsort(i[eq], kind="stable")]
        final = np.empty(EC, np.int64)
        for w in range(SW):
            wed = order[2048 * w:2048 * (w + 1)]
            nds = i[wed]
            lo, hi = 512 * w - 256, 512 * w + 768
            assert nds.min() >= lo and nds.max() < hi, (q, w, nds.min(), nds.max())
            ws = trip_cnt[wed]
            srt = np.argsort(-ws, kind="stable")
            binw = np.zeros(NT)
            binfill = [[] for _ in range(NT)]
            for ei in srt:
                cand = min((binw[bi], bi) for bi in range(NT)
                           if len(binfill[bi]) < 128)[1]
                binfill[cand].append(wed[ei])
                binw[cand] += ws[ei]
            final[2048 * w:2048 * (w + 1)] = np.concatenate(
                [np.array(bq, np.int64) for bq in binfill])
        edge_at[q] = final
        slot_of[final] = np.arange(EC)

    pos_g = (np.arange(E) // EC) * EC + slot_of
    grow = (pos_g >> 1).astype(np.int64)
    gpar = (pos_g & 1).astype(np.float32)

    # --- triplet grouping per core quarter ---
    owner_t = idx_ji // EC
    per_core = []
    ktmax = 0
    for q in range(4):
        tl = np.where(owner_t == q)[0]
        s = slot_of[idx_ji[tl]]
        w = s >> 7
        o = np.argsort(w, kind="stable")
        tl, s, w = tl[o], s[o], w[o]
        cnts = np.bincount(w, minlength=NWIN)
        ktmax = max(ktmax, int(np.ceil(cnts.max() / 128)))
        per_core.append((tl, s, cnts))
    KT = ktmax
    TPW = KT * 128
    TPAD = NWIN * TPW

    plans = []
    for q in range(4):
        tl, s, cnts = per_core[q]
        trip_slot = np.full((NWIN, TPW), -1, np.int64)
        dstf = np.full((NWIN, TPW), -1.0, np.float32)
        offs = np.zeros(NWIN + 1, np.int64)
        offs[1:] = np.cumsum(cnts)
        for wi in range(NWIN):
            c = cnts[wi]
            trip_slot[wi, :c] = tl[offs[wi]:offs[wi + 1]]
            dstf[wi, :c] = (s[offs[wi]:offs[wi + 1]] & 127).astype(np.float32)
        valid = trip_slot >= 0
        src = np.where(valid, idx_kj[np.maximum(trip_slot, 0)], 0)
        growq = np.where(valid, grow[src], 0).astype(np.int16)
        parq = np.where(valid, gpar[src], 0.0).astype(np.float32)

        gw = np.zeros((NWIN, 128, KT * 8), np.int16)
        ii = np.arange(TPW)
        prow, col = ii % 16, ii // 16
        for g in range(8):
            gw[:, g * 16 + prow, col] = growq
        dst_pk = dstf.reshape(NWIN, KT, 128).transpose(0, 2, 1).copy()
        par_pk = parq.reshape(NWIN, KT, 128).transpose(0, 2, 1).astype(bf16)

        eseq = edge_at[q]
        dstN = np.empty((SW, 128, NT), np.float32)
        for w in range(SW):
            sl = eseq[2048 * w:2048 * (w + 1)].reshape(NT, 128)
            dstN[w] = (i[sl] - (512 * w - 256)).astype(np.float32).T
        plans.append(dict(trip_slot=trip_slot, gw=gw, dst_pk=dst_pk,
                          par_pk=par_pk, dstN=dstN, eseq=eseq))

    p = params
    freq = _np(p["freq"]).astype(np.float32)
    a_env = -(P_ENV + 1) * (P_ENV + 2) / 2.0
    b_env = float(P_ENV * (P_ENV + 2))
    c_env = -P_ENV * (P_ENV + 1) / 2.0

    def W(lin):
        return _np(lin["W"]).astype(np.float32)

    def bvec(lin):
        return _np(lin["b"]).astype(np.float32)[:, None]

    pe = p["emb"]
    Wlin = W(pe["lin"])
    Wemb = W(pe["emb"])
    bemb = _np(pe["emb"]["b"]).astype(np.float32)
    u_a, u_b = Wemb @ Wlin[0:128], Wemb @ Wlin[128:256]      # [1,128]
    wts = dict(
        w_c=Wlin[256:384],
        b_comb=(bemb @ (Wlin[0:128] + Wlin[128:256])
                + _np(pe["lin"]["b"]).astype(np.float32))[:, None],
        w_rbfe=W(pe["rbf"]), b_rbfe=bvec(pe["rbf"]),
    )
    for l, blk in enumerate(p["blocks"]):
        wts[f"b{l}_wji"] = W(blk["ji"]); wts[f"b{l}_bji"] = bvec(blk["ji"])
        wts[f"b{l}_wkj"] = W(blk["kj"]); wts[f"b{l}_bkj"] = bvec(blk["kj"])
        wts[f"b{l}_w12r"] = W(blk["rbf1"]) @ W(blk["rbf2"])
        wts[f"b{l}_w12s"] = (W(blk["sbf1"]) @ W(blk["sbf2"])).astype(bf16)
        wts[f"b{l}_wdn"] = W(blk["down"])
        wts[f"b{l}_wup"] = W(blk["up"])
        wts[f"b{l}_wl1"] = W(blk["before"][0]["l1"]); wts[f"b{l}_bl1"] = bvec(blk["before"][0]["l1"])
        wts[f"b{l}_wl2"] = W(blk["before"][0]["l2"]); wts[f"b{l}_bl2"] = bvec(blk["before"][0]["l2"])
        wts[f"b{l}_wa1"] = W(blk["after"][0]["l1"]); wts[f"b{l}_ba1"] = bvec(blk["after"][0]["l1"])
        wts[f"b{l}_wa2"] = W(blk["after"][0]["l2"]); wts[f"b{l}_ba2"] = bvec(blk["after"][0]["l2"])
        wts[f"b{l}_wli"] = W(blk["lin"]); wts[f"b{l}_bli"] = bvec(blk["lin"])
    for o, ob in enumerate(p["outs"]):
        wts[f"o{o}_wr"] = W(ob["rbf"])
        wts[f"o{o}_wup"] = W(ob["up"])
        for k, lq in enumerate(ob["lins"]):
            wts[f"o{o}_wl{k}"] = W(lq)
            wts[f"o{o}_bl{k}"] = bvec(lq)
        wts[f"o{o}_wf"] = W(ob["lin"])

    in_maps = []
    for c in range(8):
        b, q = c // 4, c % 4
        pl = plans[q]
        eseq = pl["eseq"]
        d = dist[b, eseq] / CUTOFF
        env = 1.0 / d + a_env * d ** 5 + b_env * d ** 6 + c_env * d ** 7
        rbf = env[:, None] * np.sin(freq[None, :] * d[:, None])   # [EC, 6]
        x_i = x[b, i[eseq], 0]
        x_j = x[b, j[eseq], 0]
        h0pre = (u_a.T @ x_i[None, :] + u_b.T @ x_j[None, :])     # [128, EC]
        m = dict(
            rbfT=np.ascontiguousarray(rbf.T).astype(np.float32),
            h0pre=h0pre.astype(np.float32),
            gidx=pl["gw"].copy(),
            dstT=pl["dst_pk"].copy(),
            parT=pl["par_pk"].copy(),
            dstN=pl["dstN"].copy(),
            iota=np.broadcast_to(
                np.arange(1024, dtype=np.float32), (128, 1024)).copy(),
        )
        tv = pl["trip_slot"].reshape(-1)
        sb = np.zeros((TPAD, NS * NR), np.float32)
        msk = tv >= 0
        sb[msk] = sbf[b, tv[msk]]
        m["sbfT"] = np.ascontiguousarray(sb.T).astype(bf16)
        m.update(wts)
        in_maps.append(m)

    meta = dict(KT=KT, TPW=TPW, TPAD=TPAD, edge_at=edge_at, slot_of=slot_of,
                plans=plans)
    return in_maps, meta


# ----------------------------------------------------------------------------
# device program
# ----------------------------------------------------------------------------

def _build(KT, part):
    TPW = KT * 128
    TPAD = NWIN * TPW
    nc = bacc.Bacc("TRN2", target_bir_lowering=False, debug=False,
                   num_devices=8)

    def din(name, shape, dt):
        return nc.dram_tensor(name, shape, dt, kind="ExternalInput").ap()

    sbfT = din("sbfT", [NS * NR, TPAD], BF16)
    gidx = din("gidx", [NWIN, 128, KT * 8], I16)
    dstT = din("dstT", [NWIN, 128, KT], F32)
    parT = din("parT", [NWIN, 128, KT], BF16)
    dstN = din("dstN", [SW, 128, NT], F32)
    rbfT = din("rbfT", [NR, EC], F32R)
    if part == 1:
        h0pre = din("h0pre", [HC, EC], F32)
    iota = din("iota", [128, 1024], F32)

    wd = {}
    for nm, shape, dt in [
        ("w_c", [HC, HC], F32R), ("b_comb", [HC, 1], F32),
        ("w_rbfe", [NR, HC], F32R), ("b_rbfe", [HC, 1], F32),
    ]:
        wd[nm] = din(nm, shape, dt)
    for l in range(NB):
        for nm, shape, dt in [
            ("wji", [HC, HC], F32R), ("bji", [HC, 1], F32),
            ("wkj", [HC, HC], F32R), ("bkj", [HC, 1], F32),
            ("w12r", [NR, HC], F32R), ("w12s", [NS * NR, IE], BF16),
            ("wdn", [HC, IE], F32R), ("wup", [IE, HC], F32R),
            ("wl1", [HC, HC], F32R), ("bl1", [HC, 1], F32),
            ("wl2", [HC, HC], F32R), ("bl2", [HC, 1], F32),
            ("wa1", [HC, HC], F32R), ("ba1", [HC, 1], F32),
            ("wa2", [HC, HC], F32R), ("ba2", [HC, 1], F32),
            ("wli", [HC, HC], F32R), ("bli", [HC, 1], F32),
        ]:
            wd[f"b{l}_{nm}"] = din(f"b{l}_{nm}", shape, dt)
    for o in range(NB + 1):
        wd[f"o{o}_wr"] = din(f"o{o}_wr", [NR, HC], F32R)
        wd[f"o{o}_wup"] = din(f"o{o}_wup", [HC, OE], F32R)
        for k in range(3):
            wd[f"o{o}_wl{k}"] = din(f"o{o}_wl{k}", [OE, OE], F32R)
            wd[f"o{o}_bl{k}"] = din(f"o{o}_bl{k}", [OE, 1], F32)
        wd[f"o{o}_wf"] = din(f"o{o}_wf", [OE, 1], F32R)

    if part == 1:
        h_out = nc.dram_tensor("h_out", [HC, EC], F32R,
                               kind="ExternalOutput").ap()
        node_out = nc.dram_tensor("node_out", [4, 3, HC, 1024], F32,
                                  kind="ExternalOutput").ap()
    else:
        h_in = nc.dram_tensor("h_in", [HC, EC], F32R,
                              kind="ExternalInput").ap()
        nacc_in = nc.dram_tensor("nacc_in", [4, 3, HC, 1024], F32,
                                 kind="ExternalInput").ap()
        P_out = nc.dram_tensor("P_out", [1, 1024], F32,
                               kind="ExternalOutput").ap()
    dbg_out = None
    if DBG:
        dbg_out = nc.dram_tensor("dbg", [HC, EC], F32R,
                                 kind="ExternalOutput").ap()

    _lr = range(0, 2) if part == 1 else range(2, 4)
    xkj_bnc = {l: nc.dram_tensor(f"xkjb{l}", [EC, IE], BF16,
                                 kind="Internal").ap() for l in _lr}
    tables = {l: nc.dram_tensor(f"tab{l}", [E // 2, 2 * IE], BF16,
                                kind="Internal").ap() for l in _lr}
    if part == 1:
        node_bnc = node_out
    else:
        node_bnc = nc.dram_tensor("nodeball", [4, NB + 1, HC, 1024], F32,
                                  kind="Internal").ap()
        rs_out = nc.dram_tensor("rsall", [NB + 1, HC, 1024], F32,
                                kind="Internal").ap()

    GROUPS = [[0, 1, 2, 3], [4, 5, 6, 7]]

    with tile.TileContext(nc) as tc, \
         tc.tile_pool(name="cst", bufs=1) as cst, \
         tc.tile_pool(name="big", bufs=1) as big, \
         tc.tile_pool(name="mlp", bufs=1) as mlp, \
         tc.tile_pool(name="ow", bufs=1) as ow, \
         tc.tile_pool(name="bwp", bufs=2) as bwp, \
         tc.tile_pool(name="wk", bufs=2) as wk, \
         tc.tile_pool(name="tp", bufs=2) as tp, \
         tc.tile_pool(name="tp3", bufs=3) as tp3, \
         tc.tile_pool(name="ps", bufs=2, space="PSUM") as ps, \
         tc.tile_pool(name="pst_p", bufs=1, space="PSUM") as pst_p, \
         tc.tile_pool(name="psg_p", bufs=1, space="PSUM") as psg_p, \
         tc.tile_pool(name="psx_p", bufs=1, space="PSUM") as psx_p:

        iota_t = cst.tile([128, 1024], F32)
        nc.sync.dma_start(iota_t[:], iota[:])
        ident = cst.tile([128, 128], F32)
        make_identity(nc, ident[:])
        ident_bf = cst.tile([128, 128], BF16)
        nc.vector.tensor_copy(ident_bf[:], ident[:])

        wt = {}
        for nm, ap in wd.items():
            if nm.startswith("o") or (nm[0] == "b" and nm[1].isdigit()):
                continue
            t = cst.tile(list(ap.shape), ap.dtype, tag=f"w_{nm}")
            nc.sync.dma_start(t[:], ap[:])
            wt[nm] = t
        wd_b = {nm: ap for nm, ap in wd.items()
                if nm[0] == "b" and nm[1].isdigit()}

        def load_block_weights(l):
            bw = {}
            for nm, ap in wd_b.items():
                if not nm.startswith(f"b{l}_"):
                    continue
                short = nm.split("_", 1)[1]
                t = bwp.tile(list(ap.shape), ap.dtype, tag=f"bw_{short}")
                nc.sync.dma_start(t[:], ap[:])
                bw[nm] = t
            return bw

        h = big.tile([128, EC], F32R, tag="h")
        node_ext = big.tile([128, 4608], F32, tag="node_ext")
        if part == 2:
            P_acc = big.tile([1, 1024], F32, tag="P_acc")
            nc.gpsimd.memset(P_acc[:], 0.0)

        def cs(c):
            return slice(512 * c, 512 * (c + 1))

        def load_rbf(c):
            r = wk.tile([NR, 512], F32R, tag="rbf_c")
            nc.sync.dma_start(r[:], rbfT[:, cs(c)])
            return r

        # ---- embedding (part 1) / h load (part 2) ----
        if part == 2:
            nc.sync.dma_start(h[:], h_in[:])
        for c in range(NCH if part == 1 else 0):
            rbc = load_rbf(c)
            psr = ps.tile([128, 512], F32, tag="psA")
            nc.tensor.matmul(psr[:], wt["w_rbfe"][:], rbc[:],
                             start=True, stop=True)
            rh = wk.tile([128, 512], F32R, tag="pA")
            nc.scalar.activation(rh[:], psr[:], SILU, bias=wt["b_rbfe"][:])
            ps2 = ps.tile([128, 512], F32, tag="psB")
            nc.tensor.matmul(ps2[:], wt["w_c"][:], rh[:],
                             start=True, stop=True)
            h0c = wk.tile([128, 512], F32, tag="pB")
            nc.sync.dma_start(h0c[:], h0pre[:, cs(c)])
            tsum = wk.tile([128, 512], F32, tag="pC")
            nc.vector.tensor_add(tsum[:], h0c[:], ps2[:])
            nc.scalar.activation(h[:, cs(c)], tsum[:], SILU,
                                 bias=wt["b_comb"][:])

        # ---- output block ----
        def out_block(o):
            own = {}
            t = ow.tile([NR, HC], F32R, tag="ow_wr")
            nc.sync.dma_start(t[:], wd[f"o{o}_wr"][:])
            own["wr"] = t
            nc.gpsimd.memset(node_ext[:], 0.0)
            for w in range(SW):
                nx = psx_p.tile([128, 1024], F32, tag="psX")
                na = nx[:, 0:512]
                nb_ = nx[:, 512:1024]
                dn = wk.tile([128, NT], F32, tag="dn")
                nc.sync.dma_start(dn[:], dstN[w])
                for c4 in range(4):
                    c = 4 * w + c4
                    rbc = load_rbf(c)
                    pso = ps.tile([128, 512], F32, tag="psA")
                    nc.tensor.matmul(pso[:], own["wr"][:], rbc[:],
                                     start=True, stop=True)
                    tb = wk.tile([128, 512], F32, tag="pD")
                    nc.vector.tensor_tensor(tb[:], h[:, cs(c)], pso[:], op=MUL)
                    for k4 in range(4):
                        k = 4 * c4 + k4
                        pst = pst_p.tile([128, 128], F32, tag="psT")
                        nc.tensor.transpose(
                            out=pst[:], in_=tb[:, 128 * k4:128 * (k4 + 1)],
                            identity=ident[:])
                        tt = wk.tile([128, 128], F32R, tag="tt")
                        nc.vector.tensor_copy(tt[:], pst[:])
                        Sn = wk.tile([128, 1024], F32R, tag="Sn")
                        nc.vector.tensor_tensor(
                            Sn[:], dn[:, k:k + 1].to_broadcast([128, 1024]),
                            iota_t[:], op=EQ)
                        nc.tensor.matmul(na, tt[:], Sn[:, 0:512],
                                         start=(k == 0), stop=(k == NT - 1))
                        nc.tensor.matmul(nb_, tt[:], Sn[:, 512:1024],
                                         start=(k == 0), stop=(k == NT - 1))
                nc.vector.tensor_add(node_ext[:, 512 * w:512 * (w + 1)],
                                     node_ext[:, 512 * w:512 * (w + 1)], na)
                nc.vector.tensor_add(node_ext[:, 512 * (w + 1):512 * (w + 2)],
                                     node_ext[:, 512 * (w + 1):512 * (w + 2)],
                                     nb_)
            for qq in range(4):
                nc.sync.dma_start(
                    node_bnc[qq, o],
                    node_ext[:, 256 + 1024 * qq:256 + 1024 * (qq + 1)])

        def mlp_block(o):
            own = {}
            for nm, shape in [("wup", [HC, OE]),
                              ("wl0", [OE, OE]), ("bl0", [OE, 1]),
                              ("wl1", [OE, OE]), ("bl1", [OE, 1]),
                              ("wl2", [OE, OE]), ("bl2", [OE, 1]),
                              ("wf", [OE, 1])]:
                ap = wd[f"o{o}_{nm}"]
                if shape[0] > 128:
                    t = ow.tile([128, 2] + shape[1:], ap.dtype, tag=f"ow_{nm}")
                    nc.sync.dma_start(
                        t[:], ap[:].rearrange("(a p) x -> p a x", p=128))
                else:
                    t = ow.tile(shape, ap.dtype, tag=f"ow_{nm}")
                    nc.sync.dma_start(t[:], ap[:])
                own[nm] = t
            u0r = mlp.tile([128, 1024], F32, tag="u0r")
            nc.sync.dma_start(u0r[:], rs_out[o])
            u0 = mlp.tile([128, 1024], F32R, tag="u0")
            nc.vector.tensor_copy(u0[:], u0r[:])
            u1 = mlp.tile([128, 2, 1024], F32R, tag="uA")
            for mo in range(2):
                for nn in range(2):
                    psu = ps.tile([128, 512], F32, tag="psA")
                    nc.tensor.matmul(psu[:],
                                     own["wup"][:, mo * 128:(mo + 1) * 128],
                                     u0[:, 512 * nn:512 * (nn + 1)],
                                     start=True, stop=True)
                    nc.vector.tensor_copy(u1[:, mo, 512 * nn:512 * (nn + 1)],
                                          psu[:])
            uprev = u1
            for k in range(3):
                unext = mlp.tile([128, 2, 1024], F32R,
                                 tag="uB" if k % 2 == 0 else "uA")
                wl = own[f"wl{k}"]
                bl = own[f"bl{k}"]
                for mo in range(2):
                    for nn in range(2):
                        psu = ps.tile([128, 512], F32, tag="psA")
                        nc.tensor.matmul(
                            psu[:], wl[:, 0, mo * 128:(mo + 1) * 128],
                            uprev[:, 0, 512 * nn:512 * (nn + 1)],
                            start=True, stop=False)
                        nc.tensor.matmul(
                            psu[:], wl[:, 1, mo * 128:(mo + 1) * 128],
                            uprev[:, 1, 512 * nn:512 * (nn + 1)],
                            start=False, stop=True)
                        nc.scalar.activation(
                            unext[:, mo, 512 * nn:512 * (nn + 1)], psu[:],
                            SILU, bias=bl[:, mo, :])
                uprev = unext
            wf = own["wf"]
            for nn in range(2):
                psf = psg_p.tile([1, 512], F32, tag="psG")
                nc.tensor.matmul(psf[:], wf[:, 0, :],
                                 uprev[:, 0, 512 * nn:512 * (nn + 1)],
                                 start=True, stop=False)
                nc.tensor.matmul(psf[:], wf[:, 1, :],
                                 uprev[:, 1, 512 * nn:512 * (nn + 1)],
                                 start=False, stop=True)
                nc.vector.tensor_add(P_acc[:, 512 * nn:512 * (nn + 1)],
                                     P_acc[:, 512 * nn:512 * (nn + 1)],
                                     psf[:])

        if part == 1:
            out_block(0)

        # ---- interaction blocks ----
        for l in (range(0, 2) if part == 1 else range(2, 4)):
            pre = f"b{l}_"
            wt_b = load_block_weights(l)
            wt.update(wt_b)
            for c in range(NCH):
                psk = ps.tile([128, 512], F32, tag="psA")
                nc.tensor.matmul(psk[:], wt[pre + "wkj"][:], h[:, cs(c)],
                                 start=True, stop=True)
                xkj = wk.tile([128, 512], F32R, tag="pA")
                nc.scalar.activation(xkj[:], psk[:], SILU,
                                     bias=wt[pre + "bkj"][:])
                rbc = load_rbf(c)
                psr = ps.tile([128, 512], F32, tag="psB")
                nc.tensor.matmul(psr[:], wt[pre + "w12r"][:], rbc[:],
                                 start=True, stop=True)
                xkr = wk.tile([128, 512], F32R, tag="pB")
                nc.vector.tensor_tensor(xkr[:], xkj[:], psr[:], op=MUL)
                psd = ps.tile([64, 512], F32, tag="psA")
                nc.tensor.matmul(psd[:], wt[pre + "wdn"][:], xkr[:],
                                 start=True, stop=True)
                xd = wk.tile([64, 512], BF16, tag="xd")
                nc.scalar.activation(xd[:], psd[:], SILU)
                pst = pst_p.tile([128, 256], BF16, tag="psT")
                for k in range(4):
                    nc.tensor.transpose(
                        out=pst[:, 64 * k:64 * (k + 1)],
                        in_=xd[:, 128 * k:128 * (k + 1)],
                        identity=ident_bf[:64, :64])
                tb4 = wk.tile([128, 4, 64], BF16, tag="tb4")
                nc.vector.tensor_copy(
                    tb4[:], pst[:].rearrange("p (a x) -> p a x", a=4))
                for a in range(4):
                    nc.sync.dma_start(
                        xkj_bnc[l][512 * c + 128 * a:512 * c + 128 * (a + 1), :],
                        tb4[:, a, :])
            nc.gpsimd.collective_compute(
                "AllGather", mybir.AluOpType.bypass, replica_groups=GROUPS,
                ins=[xkj_bnc[l].opt()], outs=[tables[l].opt()])

            agg_c = None
            for w in range(NWIN):
                gix = tp.tile([128, KT * 8], I16, tag="gix")
                nc.sync.dma_start(gix[:], gidx[w])
                dt_ = tp.tile([128, KT], F32, tag="dt_")
                nc.sync.dma_start(dt_[:], dstT[w])
                pt_ = tp.tile([128, KT], BF16, tag="pt_")
                nc.sync.dma_start(pt_[:], parT[w])
                sfl = tp3.tile([NS * NR, TPW], BF16, tag="sfl")
                nc.sync.dma_start(sfl[:], sbfT[:, TPW * w:TPW * (w + 1)])
                gt = tp3.tile([128, KT, 2 * IE], BF16, tag="gt")
                if NOGATHER:
                    nc.gpsimd.memset(gt[:], 0.0)
                else:
                    nc.gpsimd.dma_gather(
                        out_ap=gt[:], in_ap=tables[l][:], idxs_ap=gix[:],
                        num_idxs=TPW, num_idxs_reg=TPW, elem_size=2 * IE,
                        single_packet=False)
                psex = psx_p.tile([128, 1024], F32, tag="psX")
                pse = psex[:, 0:KT * IE]
                for k in range(KT):
                    nc.tensor.matmul(psex[:, IE * k:IE * (k + 1)],
                                     sfl[:, 128 * k:128 * (k + 1)],
                                     wt[pre + "w12s"][:],
                                     start=True, stop=True)
                sfe = tp.tile([128, KT, IE], BF16, tag="sfe")
                nc.vector.tensor_copy(
                    sfe[:], pse.rearrange("p (a x) -> p a x", a=KT))
                gd = tp.tile([128, KT, IE], BF16, tag="gd")
                nc.vector.tensor_tensor(gd[:], gt[:, :, IE:2 * IE],
                                        gt[:, :, 0:IE], op=SUB)
                nc.vector.tensor_tensor(
                    gd[:], gd[:],
                    pt_[:, :, None].to_broadcast([128, KT, IE]), op=MUL)
                nc.vector.tensor_add(gd[:], gd[:], gt[:, :, 0:IE])
                nc.vector.tensor_tensor(gd[:], gd[:], sfe[:], op=MUL)
                Sa = tp.tile([128, KT, 128], BF16, tag="Sa")
                nc.vector.tensor_tensor(
                    Sa[:], dt_[:, :, None].to_broadcast([128, KT, 128]),
                    iota_t[:, None, 0:128].to_broadcast([128, KT, 128]),
                    op=EQ)
                pag = psg_p.tile([64, 128], F32, tag="psG")
                for k in range(KT):
                    nc.tensor.matmul(pag[:], gd[:, k, :], Sa[:, k, :],
                                     start=(k == 0), stop=(k == KT - 1))
                if w % 4 == 0:
                    agg_c = wk.tile([64, 512], F32R, tag="agg_c")
                nc.vector.tensor_copy(
                    agg_c[:, 128 * (w % 4):128 * (w % 4 + 1)], pag[:])
                if w % 4 == 3:
                    c = w // 4
                    psu = ps.tile([128, 512], F32, tag="psA")
                    nc.tensor.matmul(psu[:], wt[pre + "wup"][:], agg_c[:],
                                     start=True, stop=True)
                    t1 = wk.tile([128, 512], F32R, tag="pA")
                    nc.scalar.activation(t1[:], psu[:], SILU)
                    psj = ps.tile([128, 512], F32, tag="psB")
                    nc.tensor.matmul(psj[:], wt[pre + "wji"][:], h[:, cs(c)],
                                     start=True, stop=True)
                    t2 = wk.tile([128, 512], F32R, tag="pB")
                    nc.scalar.activation(t2[:], psj[:], SILU,
                                         bias=wt[pre + "bji"][:])
                    hh = wk.tile([128, 512], F32R, tag="pC")
                    nc.vector.tensor_add(hh[:], t1[:], t2[:])
                    ps1 = ps.tile([128, 512], F32, tag="psA")
                    nc.tensor.matmul(ps1[:], wt[pre + "wl1"][:], hh[:],
                                     start=True, stop=True)
                    r1 = wk.tile([128, 512], F32R, tag="pA")
                    nc.scalar.activation(r1[:], ps1[:], SILU,
                                         bias=wt[pre + "bl1"][:])
                    ps2_ = ps.tile([128, 512], F32, tag="psB")
                    nc.tensor.matmul(ps2_[:], wt[pre + "wl2"][:], r1[:],
                                     start=True, stop=True)
                    r2 = wk.tile([128, 512], F32R, tag="pB")
                    nc.scalar.activation(r2[:], ps2_[:], SILU,
                                         bias=wt[pre + "bl2"][:])
                    nc.vector.tensor_add(hh[:], hh[:], r2[:])
                    ps3 = ps.tile([128, 512], F32, tag="psA")
                    nc.tensor.matmul(ps3[:], wt[pre + "wli"][:], hh[:],
                                     start=True, stop=True)
                    r3 = wk.tile([128, 512], F32R, tag="pA")
                    nc.scalar.activation(r3[:], ps3[:], SILU,
                                         bias=wt[pre + "bli"][:])
                    nc.vector.tensor_add(h[:, cs(c)], h[:, cs(c)], r3[:])
                    ps4 = ps.tile([128, 512], F32, tag="psB")
                    nc.tensor.matmul(ps4[:], wt[pre + "wa1"][:], h[:, cs(c)],
                                     start=True, stop=True)
                    r4 = wk.tile([128, 512], F32R, tag="pB")
                    nc.scalar.activation(r4[:], ps4[:], SILU,
                                         bias=wt[pre + "ba1"][:])
                    ps5 = ps.tile([128, 512], F32, tag="psA")
                    nc.tensor.matmul(ps5[:], wt[pre + "wa2"][:], r4[:],
                                     start=True, stop=True)
                    r5 = wk.tile([128, 512], F32R, tag="pC2")
                    nc.scalar.activation(r5[:], ps5[:], SILU,
                                         bias=wt[pre + "ba2"][:])
                    nc.vector.tensor_add(h[:, cs(c)], h[:, cs(c)], r5[:])

            if DBG == f"h{l}":
                nc.sync.dma_start(dbg_out[:], h[:])
            out_block(l + 1)

        if DBG == "emb":
            nc.sync.dma_start(dbg_out[:], h[:])

        if part == 1:
            nc.sync.dma_start(h_out[:], h[:])
        else:
            for qq in range(4):
                for o in range(3):
                    stg = wk.tile([128, 1024], F32, tag="stg")
                    nc.sync.dma_start(stg[:], nacc_in[qq, o])
                    nc.sync.dma_start(node_bnc[qq, o], stg[:])
            nc.gpsimd.collective_compute(
                "ReduceScatter", mybir.AluOpType.add, replica_groups=GROUPS,
                ins=[node_bnc.opt()], outs=[rs_out.opt()])
            for o in range(NB + 1):
                mlp_block(o)
            nc.sync.dma_start(P_out[:], P_acc[:])

    t0 = time.time()
    nc.compile()
    print(f"[kernel] bacc compile {time.time() - t0:.1f}s", flush=True)
    return nc


# ----------------------------------------------------------------------------
# entry point
# ----------------------------------------------------------------------------

_CACHE = {}


def kernel(x, dist, sbf, i, j, idx_kj, idx_ji, params):
    t0 = time.time()
    in_maps, meta = _host_plan(x, dist, sbf, i, j, idx_kj, idx_ji, params)
    print(f"[kernel] host plan {time.time() - t0:.1f}s (KT={meta['KT']})",
          flush=True)
    KT = meta["KT"]
    for part in (1, 2):
        if (KT, part) not in _CACHE:
            t0 = time.time()
            _CACHE[(KT, part)] = _build(KT, part)
            print(f"[kernel] build part{part} {time.time() - t0:.1f}s",
                  flush=True)
    nc1, nc2 = _CACHE[(KT, 1)], _CACHE[(KT, 2)]
    t0 = time.time()
    res1 = bass_utils.run_bass_kernel_spmd(nc1, in_maps,
                                           core_ids=list(range(8)))
    print(f"[kernel] run part1 {time.time() - t0:.1f}s", flush=True)
    in_maps2 = []
    for c in range(8):
        m = dict(in_maps[c])
        m["h_in"] = res1.results[c]["h_out"]
        m["nacc_in"] = res1.results[c]["node_out"]
        m.pop("h0pre")
        in_maps2.append(m)
    t0 = time.time()
    res = bass_utils.run_bass_kernel_spmd(nc2, in_maps2,
                                          core_ids=list(range(8)))
    print(f"[kernel] run part2 {time.time() - t0:.1f}s", flush=True)
    P = np.zeros((B, N, OC), np.float32)
    for c in range(8):
        b, q = c // 4, c % 4
        P[b, 1024 * q:1024 * (q + 1), 0] = res.results[c]["P_out"][0]
    kernel._last = (nc1, nc2, in_maps, meta, res1, res)
    return P
